# revision 1
# baseline (speedup 1.0000x reference)
"""MLA attention (DeepSeek-style) Trainium2 Bass kernel, 8-core SPMD.

Sharding: core c handles batch b = c//4 and head-group g = c%4 (4 of 16 heads).
Down-projections are replicated per batch; up-projections / attention / o-proj
are head-parallel. Host sums the 4 partial o-projections per batch.

v2 schedule (vs baseline): all activations double-buffered so chunk ic+1's
down-projections interleave into chunk ic's attention stalls; rope is
decoupled from PSUM via ScalarE drains to SBUF + bf16 DVE math; kT/vnat are
per-chunk tiles (no false cross-chunk deps); causal diagonal is trimmed at
128 granularity; psS has 3 PSUM banks so the scores->exp->attnout pipeline
runs at exp rate; small weights resident in SBUF; bf16 latents and output
partials.
"""

import numpy as np
import ml_dtypes

import concourse.bacc as bacc
import concourse.mybir as mybir
import concourse.tile as tile
from concourse.bass_utils import run_bass_kernel_spmd

F32 = mybir.dt.float32
BF16 = mybir.dt.bfloat16

B, S, D = 2, 2048, 2048
H, HD = 16, 128
RD, ND = 64, 64
KVR, QR = 512, 1024
BASE = 10000.0
HLOC = 4                 # heads per core
CHUNK = 512
NCHUNK = S // CHUNK      # 4
P = 128
SCALE = HD ** -0.5

_BF16 = ml_dtypes.bfloat16


def _build():
    nc = bacc.Bacc("TRN2", target_bir_lowering=False, debug=False)

    xT = nc.dram_tensor("xT", [D, S], BF16, kind="ExternalInput").ap()
    wqd = nc.dram_tensor("wqd", [D, QR], BF16, kind="ExternalInput").ap()
    wkvd = nc.dram_tensor("wkvd", [D, KVR], BF16, kind="ExternalInput").ap()
    wkr = nc.dram_tensor("wkr", [D, HLOC * RD], BF16, kind="ExternalInput").ap()
    wqcat = nc.dram_tensor("wqcat", [QR, HLOC * HD], BF16, kind="ExternalInput").ap()
    wkup = nc.dram_tensor("wkup", [KVR, HLOC * ND], BF16, kind="ExternalInput").ap()
    wvup = nc.dram_tensor("wvup", [KVR, HLOC * HD], BF16, kind="ExternalInput").ap()
    wo = nc.dram_tensor("wo", [HLOC * HD, D], BF16, kind="ExternalInput").ap()
    cosr = nc.dram_tensor("cosr", [P, S], BF16, kind="ExternalInput").ap()
    sinr = nc.dram_tensor("sinr", [P, S], BF16, kind="ExternalInput").ap()
    maskd = nc.dram_tensor("maskd", [P, P], BF16, kind="ExternalInput").ap()
    o_part = nc.dram_tensor("o_part", [S, D], BF16, kind="ExternalOutput").ap()

    xT_r = xT.rearrange("(dt p) s -> p dt s", p=P)          # [128, 16, S]
    wqd_r = wqd.rearrange("(dt p) q -> p dt q", p=P)        # [128, 16, 1024]
    wkvd_r = wkvd.rearrange("(dt p) q -> p dt q", p=P)      # [128, 16, 512]
    wkr_r = wkr.rearrange("(dt p) q -> p dt q", p=P)        # [128, 16, 256]
    wqcat_r = wqcat.rearrange("(qt p) c -> p qt c", p=P)    # [128, 8, 512]
    wkup_r = wkup.rearrange("(kt p) c -> p kt c", p=P)      # [128, 4, 256]
    wvup_r = wvup.rearrange("(kt p) c -> p kt c", p=P)      # [128, 4, 512]
    wo_r = wo.rearrange("(kt p) d -> p kt d", p=P)          # [128, 4, 2048]
    o_r = o_part.rearrange("(st p) d -> p st d", p=P)       # [128, 16, 2048]

    with tile.TileContext(nc) as tc:
        with (
            tc.tile_pool(name="persist", bufs=1) as pp,
            tc.tile_pool(name="acts", bufs=2) as ap_,
            tc.tile_pool(name="wstream", bufs=3) as wp,
            tc.tile_pool(name="rope", bufs=2) as rp,
            tc.tile_pool(name="attn", bufs=3) as atp,
            tc.tile_pool(name="recp", bufs=2) as rcp,
            tc.tile_pool(name="outp", bufs=2) as op_,
            tc.tile_pool(name="aoutp", bufs=2) as aop,
            tc.tile_pool(name="psA", bufs=2, space="PSUM") as psA,
            tc.tile_pool(name="psS", bufs=3, space="PSUM") as psS,
            tc.tile_pool(name="psD", bufs=2, space="PSUM") as psD,
            tc.tile_pool(name="psO", bufs=1, space="PSUM") as psO,
        ):
            # ---------------- persistent tiles ----------------
            kT = [pp.tile([P, HLOC, CHUNK], BF16, name=f"kT{j}", tag=f"kT{j}")
                  for j in range(NCHUNK)]                     # per-chunk K^T
            vnat = [pp.tile([P, CHUNK // P, HLOC * HD], BF16, name=f"vn{j}", tag=f"vn{j}")
                    for j in range(NCHUNK)]                   # per-chunk V nat
            mask = pp.tile([P, P], BF16, tag="mask")
            ones = pp.tile([P, P], BF16, tag="ones")
            wo_t = pp.tile([P, HLOC, D], BF16, tag="wo")
            wqc_t = pp.tile([P, QR // P, HLOC * HD], BF16, tag="wqc")
            wku_t = pp.tile([P, KVR // P, HLOC * ND], BF16, tag="wku")
            wvu_t = pp.tile([P, KVR // P, HLOC * HD], BF16, tag="wvu")
            wkr_t = pp.tile([P, D // P, HLOC * RD], BF16, tag="wkr")
            cos_t = pp.tile([P, S], BF16, tag="cos")
            sin_t = pp.tile([P, S], BF16, tag="sin")

            nc.vector.memset(ones[:], 1.0)
            # PE warm-up during the initial DMA ramp: ~3.5us of tiny matmuls
            # (only dep: the memset) un-throttle the HAM clock gate before the
            # first real matmul arrives
            wps = psA.tile([P, CHUNK], F32, name="warmps", tag="psA")
            for _ in range(155):
                nc.tensor.matmul(wps[0:64, 0:64], ones[:, 0:64],
                                 ones[:, 0:64], start=True, stop=True)

            def load_residents():
                """Emitted after chunk 0's critical x/weight DMAs so these
                don't compete for HBM bandwidth before the first matmul;
                ordered by first use."""
                nc.sync.dma_start(mask[:], maskd[:])
                nc.sync.dma_start(cos_t[:], cosr[:])
                nc.sync.dma_start(sin_t[:], sinr[:])
                nc.sync.dma_start(wkr_t[:], wkr_r[:])
                nc.sync.dma_start(wqc_t[:], wqcat_r[:])
                nc.sync.dma_start(wku_t[:], wkup_r[:])
                nc.sync.dma_start(wvu_t[:], wvup_r[:])
                for kt_ in range(HLOC):
                    nc.sync.dma_start(wo_t[:, kt_, :], wo_r[:, kt_, :])

            def o_proj(ic, aout, sts=range(CHUNK // P), final=False):
                """Project previous chunk's attention output; PE filler work
                staged across the next chunk's attention loop."""
                for st in sts:
                    osb = op_.tile([P, D], BF16, tag="osb")
                    for dc in range(D // CHUNK):
                        ps = psA.tile([P, CHUNK], F32, tag="psA")
                        for kt_ in range(HLOC):
                            nc.tensor.matmul(
                                ps[:], aout[:, kt_, P * st:P * (st + 1)],
                                wo_t[:, kt_, CHUNK * dc:CHUNK * (dc + 1)],
                                start=(kt_ == 0), stop=(kt_ == HLOC - 1))
                        if final and dc % 2 == 1:
                            # ScalarE is idle after the last exp
                            nc.scalar.copy(
                                osb[:, CHUNK * dc:CHUNK * (dc + 1)], ps[:])
                        else:
                            nc.vector.tensor_copy(
                                osb[:, CHUNK * dc:CHUNK * (dc + 1)], ps[:])
                        # per-dc store, alternated across two DMA rings so
                        # the final output drain runs on both concurrently
                        eng = nc.gpsimd if dc % 2 == 0 else nc.sync
                        eng.dma_start(
                            o_r[:, ic * (CHUNK // P) + st,
                                CHUNK * dc:CHUNK * (dc + 1)],
                            osb[:, CHUNK * dc:CHUNK * (dc + 1)])

            # ---------------- chunk loop ----------------
            for ic in range(NCHUNK):
                sl = slice(ic * CHUNK, (ic + 1) * CHUNK)

                # first weight strip before the bulky x load so the first
                # matmul's operands arrive earliest
                ws0 = wp.tile([P, D // P, 2 * P], BF16, name="ws0",
                              tag="wstrip")
                nc.sync.dma_start(ws0[:, 0:8, :], wqd_r[:, 0:8, 0:2 * P])
                nc.sync.dma_start(ws0[:, 8:16, :], wqd_r[:, 8:16, 0:2 * P])
                xc = ap_.tile([P, D // P, CHUNK], BF16, tag="xc")
                for dq in range(4):
                    nc.sync.dma_start(xc[:, 4 * dq:4 * (dq + 1), :],
                                      xT_r[:, 4 * dq:4 * (dq + 1), sl])
                cos_c = cos_t[:, sl]
                sin_c = sin_t[:, sl]

                def emit_kpe():
                    # ---- k_pe: head pair a -> heads (2a, 2a+1) rope dims ----
                    # drain psum via ScalarE to SBUF bf16, rope on DVE from SBUF
                    for a in range(2):
                        ps = psA.tile([P, CHUNK], F32, tag="psA")
                        for dt_ in range(D // P):
                            nc.tensor.matmul(
                                ps[:], wkr_t[:, dt_, P * a:P * (a + 1)],
                                xc[:, dt_, :],
                                start=(dt_ == 0), stop=(dt_ == D // P - 1))
                        raw = rp.tile([P, CHUNK], BF16, tag="kraw")
                        sh = rp.tile([P, CHUNK], BF16, tag="ksh")
                        scr = rp.tile([P, CHUNK], BF16, tag="kscr")
                        nc.scalar.copy(raw[:], ps[:])
                        # NeoX rotation: shifted halves within each 64-row block
                        for b in (0, 64):
                            nc.vector.tensor_copy(sh[b:b + 32, :],
                                                  raw[b + 32:b + 64, :])
                            nc.vector.tensor_copy(sh[b + 32:b + 64, :],
                                                  raw[b:b + 32, :])
                        nc.vector.tensor_tensor(sh[:], sh[:], sin_c,
                                                mybir.AluOpType.mult)
                        nc.vector.tensor_tensor(scr[:], raw[:], cos_c,
                                                mybir.AluOpType.mult)
                        nc.vector.tensor_tensor(kT[ic][64:128, 2 * a, :],
                                                scr[0:64, :], sh[0:64, :],
                                                mybir.AluOpType.add)
                        nc.vector.tensor_tensor(kT[ic][64:128, 2 * a + 1, :],
                                                scr[64:128, :], sh[64:128, :],
                                                mybir.AluOpType.add)

                def emit_qlat():
                    # ---- q_latT [1024, CHUNK] (bf16) ----
                    qlat = ap_.tile([P, QR // P, CHUNK], BF16, tag="qlat")
                    for cp in range(QR // P // 2):          # c-tile pairs
                        if cp == 0:
                            ws = ws0
                        else:
                            ws = wp.tile([P, D // P, 2 * P], BF16, tag="wstrip")
                            nc.sync.dma_start(
                                ws[:], wqd_r[:, :, 2 * P * cp:2 * P * (cp + 1)])
                        for ci in range(2):
                            c = 2 * cp + ci
                            ps = psA.tile([P, CHUNK], F32, tag="psA")
                            for dt_ in range(D // P):
                                nc.tensor.matmul(
                                    ps[:], ws[:, dt_, P * ci:P * (ci + 1)],
                                    xc[:, dt_, :],
                                    start=(dt_ == 0), stop=(dt_ == D // P - 1))
                            nc.scalar.copy(qlat[:, c, :], ps[:])
                    return qlat

                def emit_q(qlat):
                    # ---- q heads: c-tile h = head h [nope64 | pe64] ----
                    qTi = ap_.tile([P, HLOC, CHUNK], BF16, tag="qTi")
                    for h in range(HLOC):
                        ps = psA.tile([P, CHUNK], F32, tag="psA")
                        for qt in range(QR // P):
                            nc.tensor.matmul(
                                ps[:], wqc_t[:, qt, P * h:P * (h + 1)],
                                qlat[:, qt, :],
                                start=(qt == 0), stop=(qt == QR // P - 1))
                        nc.scalar.copy(qTi[0:64, h, :], ps[0:64, :])
                        raw = rp.tile([P, CHUNK], BF16, tag="qraw")
                        sh = rp.tile([P, CHUNK], BF16, tag="qsh")
                        scr = rp.tile([P, CHUNK], BF16, tag="qscr")
                        nc.scalar.copy(raw[64:128, :], ps[64:128, :])
                        nc.vector.tensor_copy(sh[64:96, :], raw[96:128, :])
                        nc.vector.tensor_copy(sh[96:128, :], raw[64:96, :])
                        nc.vector.tensor_tensor(sh[64:128, :], sh[64:128, :],
                                                sin_c[64:128, :],
                                                mybir.AluOpType.mult)
                        nc.vector.tensor_tensor(scr[64:128, :], raw[64:128, :],
                                                cos_c[64:128, :],
                                                mybir.AluOpType.mult)
                        nc.vector.tensor_tensor(qTi[64:128, h, :],
                                                scr[64:128, :], sh[64:128, :],
                                                mybir.AluOpType.add)
                    return qTi

                def emit_kvlat():
                    # ---- kv_latT [512, CHUNK] (bf16) ----
                    kvlat = ap_.tile([P, KVR // P, CHUNK], BF16, tag="kvlat")
                    for cp in range(KVR // P // 2):
                        ws = wp.tile([P, D // P, 2 * P], BF16, tag="wstrip")
                        nc.sync.dma_start(
                            ws[:], wkvd_r[:, :, 2 * P * cp:2 * P * (cp + 1)])
                        for ci in range(2):
                            c = 2 * cp + ci
                            ps = psA.tile([P, CHUNK], F32, tag="psA")
                            for dt_ in range(D // P):
                                nc.tensor.matmul(
                                    ps[:], ws[:, dt_, P * ci:P * (ci + 1)],
                                    xc[:, dt_, :],
                                    start=(dt_ == 0), stop=(dt_ == D // P - 1))
                            nc.scalar.copy(kvlat[:, c, :], ps[:])
                    return kvlat

                if ic == 0:
                    qlat = emit_qlat()
                    kvlat = emit_kvlat()
                    load_residents()
                    emit_kpe()
                    qTi = emit_q(qlat)
                else:
                    emit_kpe()
                    qlat = emit_qlat()
                    qTi = emit_q(qlat)
                    kvlat = emit_kvlat()

                # ---- k_nope: head pair a -> heads (2a, 2a+1) nope dims ----
                for a in range(2):
                    ps = psA.tile([P, CHUNK], F32, tag="psA")
                    for kt_ in range(KVR // P):
                        nc.tensor.matmul(
                            ps[:], wku_t[:, kt_, P * a:P * (a + 1)],
                            kvlat[:, kt_, :],
                            start=(kt_ == 0), stop=(kt_ == KVR // P - 1))
                    if ic == 0:
                        # chunk 0's window is DVE-bound (rope chain); ScalarE
                        # has slack there (no overlapping attention exp)
                        nc.scalar.copy(kT[ic][0:64, 2 * a, :], ps[0:64, :])
                        nc.scalar.copy(kT[ic][0:64, 2 * a + 1, :],
                                       ps[64:128, :])
                    else:
                        nc.vector.tensor_copy(kT[ic][0:64, 2 * a, :],
                                              ps[0:64, :])
                        nc.vector.tensor_copy(kT[ic][0:64, 2 * a + 1, :],
                                              ps[64:128, :])

                # ---- v natural [CHUNK, 512] ----
                for st in range(CHUNK // P):
                    ps = psA.tile([P, HLOC * HD], F32, tag="psA")
                    for kt_ in range(KVR // P):
                        nc.tensor.matmul(
                            ps[:], kvlat[:, kt_, P * st:P * (st + 1)],
                            wvu_t[:, kt_, :],
                            start=(kt_ == 0), stop=(kt_ == KVR // P - 1))
                    if ic == 0:
                        nc.scalar.copy(vnat[ic][:, st, :], ps[:])
                    else:
                        nc.vector.tensor_copy(vnat[ic][:, st, :], ps[:])

                # ---- o-projection of the PREVIOUS chunk: PE filler that
                # covers this chunk's rope/DVE latency before attention
                if ic > 0:
                    o_proj(ic - 1, prev_aout, sts=(0, 1))

                # ---- attention for this query chunk (diagonal trimmed) ----
                aout = aop.tile([P, HLOC, CHUNK], BF16, tag="aout")
                for h in range(HLOC):
                    if ic > 0 and h in (2, 3):
                        # reserve late PE filler for the exp-paced tail
                        o_proj(ic - 1, prev_aout, sts=(h,))
                    psd = psD.tile([P, CHUNK], F32, tag="psD")
                    pso = psO.tile([P, CHUNK], F32, tag="psO")
                    nj = 4 * ic + 4            # total j-tiles incl. diagonal
                    for jt in range(nj):
                        jc, r = divmod(jt, 4)
                        diag = jc == ic
                        off = P * r if diag else 0      # first query col
                        n = CHUNK - off
                        first, last = jt == 0, jt == nj - 1
                        pss = psS.tile([P, CHUNK], F32, tag="psS")
                        nc.tensor.matmul(
                            pss[:, off:], kT[jc][:, h, P * r:P * (r + 1)],
                            qTi[:, h, off:], start=True, stop=True)
                        at = atp.tile([P, CHUNK], BF16, tag="attnT")
                        nc.scalar.activation(
                            at[:, off:], pss[:, off:],
                            mybir.ActivationFunctionType.Exp, scale=SCALE)
                        if diag:
                            nc.vector.tensor_tensor(
                                at[:, off:off + P], at[:, off:off + P],
                                mask[:], mybir.AluOpType.mult)
                        nc.tensor.matmul(
                            pso[:, off:], vnat[jc][:, r, HD * h:HD * (h + 1)],
                            at[:, off:], start=first, stop=last)
                        nc.tensor.matmul(psd[:, off:], ones[:], at[:, off:],
                                         start=first, stop=last)
                    rec = rcp.tile([P, CHUNK], F32, tag="recip")
                    nc.vector.reciprocal_approx_fast(rec[:], psd[:])
                    if ic == NCHUNK - 1 and h == HLOC - 1:
                        # final head gates o_proj(3): normalize per query
                        # slice so its st-blocks start earlier
                        for stq in range(CHUNK // P):
                            qs = slice(P * stq, P * (stq + 1))
                            nc.vector.tensor_tensor(
                                aout[:, h, qs], pso[:, qs], rec[:, qs],
                                mybir.AluOpType.mult)
                    else:
                        nc.vector.tensor_tensor(aout[:, h, :], pso[:], rec[:],
                                                mybir.AluOpType.mult)
                prev_aout = aout

            o_proj(NCHUNK - 1, prev_aout, final=True)
    nc.compile()
    return nc


_NC = None


def _get_nc():
    global _NC
    if _NC is None:
        _NC = _build()
    return _NC


def _host_prep(x, Wq_down, Wq_up, Wq_rope, Wkv_down, Wk_up, Wk_rope, Wv_up, Wo):
    """Build the 8 per-core input maps (all host-side layout prep)."""
    # rope tables, replicated to 128 partitions with NeoX sign baked into sin
    half = RD // 2
    inv_freq = 1.0 / (BASE ** (np.arange(half, dtype=np.float64) / half))
    ang = np.arange(S, dtype=np.float64)[None, :] * inv_freq[:, None]  # [32, S]
    cos32 = np.cos(ang)
    sin32 = np.sin(ang)
    cosr = np.tile(cos32, (4, 1)).astype(_BF16)                        # [128,S]
    sinr = np.concatenate([-sin32, sin32, -sin32, sin32], 0).astype(_BF16)

    # causal mask for the 128x128 diagonal block: key p visible to query c
    pidx = np.arange(P)[:, None]
    cidx = np.arange(P)[None, :]
    maskd = (pidx <= cidx).astype(_BF16)

    xT = [np.ascontiguousarray(x[b].T).astype(_BF16) for b in range(B)]
    wqd = Wq_down.astype(_BF16)
    wkvd = Wkv_down.astype(_BF16)

    in_maps = []
    for c in range(8):
        b, g = divmod(c, 4)
        heads = range(HLOC * g, HLOC * (g + 1))
        wqcat = np.empty((QR, HLOC * HD), np.float32)
        for i, h in enumerate(heads):
            wqcat[:, i * HD:i * HD + ND] = Wq_up[:, h * ND:(h + 1) * ND]
            wqcat[:, i * HD + ND:(i + 1) * HD] = Wq_rope[:, h * RD:(h + 1) * RD]
        in_maps.append({
            "xT": xT[b],
            "wqd": wqd,
            "wkvd": wkvd,
            "wkr": np.ascontiguousarray(
                Wk_rope[:, g * HLOC * RD:(g + 1) * HLOC * RD]).astype(_BF16),
            "wqcat": wqcat.astype(_BF16),
            "wkup": np.ascontiguousarray(
                Wk_up[:, g * HLOC * ND:(g + 1) * HLOC * ND]).astype(_BF16),
            "wvup": np.ascontiguousarray(
                Wv_up[:, g * HLOC * HD:(g + 1) * HLOC * HD]).astype(_BF16),
            "wo": np.ascontiguousarray(
                Wo[g * HLOC * HD:(g + 1) * HLOC * HD, :]).astype(_BF16),
            "cosr": cosr,
            "sinr": sinr,
            "maskd": maskd,
        })
    return in_maps


def kernel(x, Wq_down, Wq_up, Wq_rope, Wkv_down, Wk_up, Wk_rope, Wv_up, Wo,
           _trace=False, _trace_kwargs=None):
    x = np.asarray(x, dtype=np.float32)
    args = [np.asarray(a, dtype=np.float32) for a in
            (Wq_down, Wq_up, Wq_rope, Wkv_down, Wk_up, Wk_rope, Wv_up, Wo)]
    in_maps = _host_prep(x, *args)
    nc = _get_nc()
    res = run_bass_kernel_spmd(nc, in_maps, core_ids=list(range(8)),
                               trace=_trace, **(_trace_kwargs or {}))
    kernel._last_results = res
    out = np.zeros((B, S, D), np.float32)
    for c in range(8):
        out[c // 4] += res.results[c]["o_part"].astype(np.float32)
    return out



# revision 4
# speedup vs baseline: 1.0374x; 1.0374x over previous
"""MLA attention (DeepSeek-style) Trainium2 Bass kernel, 8-core SPMD.

Sharding: core c handles batch b = c//4 and head-group g = c%4 (4 of 16 heads).
v3: the latent down-projections (q_lat, kv_lat) are no longer replicated across
the 4 cores of a batch group — each core computes a 1/4 column slice and the
slices are exchanged with a per-chunk AllGather over DRAM bounce buffers
(replica groups [[0..3],[4..7]]). Prologue computes all latent partials + k_pe
(both only need x) and launches the 4 gathers; the main loop consumes gathered
latents for the head-parallel up-projections / attention / o-projection.
Everything else keeps the v2 schedule: double-buffered activations, ScalarE
psum drains + bf16 DVE rope, per-chunk kT/vnat, diagonal trimmed at 128
granularity, 3 psS banks, bf16 everywhere off-psum.
"""

import numpy as np
import ml_dtypes

import concourse.bacc as bacc
import concourse.mybir as mybir
import concourse.tile as tile
from concourse.bass_utils import run_bass_kernel_spmd

F32 = mybir.dt.float32
BF16 = mybir.dt.bfloat16

B, S, D = 2, 2048, 2048
H, HD = 16, 128
RD, ND = 64, 64
KVR, QR = 512, 1024
BASE = 10000.0
HLOC = 4                 # heads per core
CHUNK = 512
NCHUNK = S // CHUNK      # 4
P = 128
SCALE = HD ** -0.5
QRL = QR // 4            # per-core q_lat slice (2 c-tiles)
KVRL = KVR // 4          # per-core kv_lat slice (1 c-tile)
GROUPS = [[0, 1, 2, 3], [4, 5, 6, 7]]

_BF16 = ml_dtypes.bfloat16


def _build():
    nc = bacc.Bacc("TRN2", target_bir_lowering=False, debug=False, num_devices=8)

    xT = nc.dram_tensor("xT", [D, S], BF16, kind="ExternalInput").ap()
    wqd = nc.dram_tensor("wqd", [D, QRL], BF16, kind="ExternalInput").ap()
    wkvd = nc.dram_tensor("wkvd", [D, KVRL], BF16, kind="ExternalInput").ap()
    wkr = nc.dram_tensor("wkr", [D, HLOC * RD], BF16, kind="ExternalInput").ap()
    wqcat = nc.dram_tensor("wqcat", [QR, HLOC * HD], BF16, kind="ExternalInput").ap()
    wkup = nc.dram_tensor("wkup", [KVR, HLOC * ND], BF16, kind="ExternalInput").ap()
    wvup = nc.dram_tensor("wvup", [KVR, HLOC * HD], BF16, kind="ExternalInput").ap()
    wo = nc.dram_tensor("wo", [HLOC * HD, D], BF16, kind="ExternalInput").ap()
    cosr = nc.dram_tensor("cosr", [P, S], BF16, kind="ExternalInput").ap()
    sinr = nc.dram_tensor("sinr", [P, S], BF16, kind="ExternalInput").ap()
    maskd = nc.dram_tensor("maskd", [P, P], BF16, kind="ExternalInput").ap()
    o_part = nc.dram_tensor("o_part", [S, D], BF16, kind="ExternalOutput").ap()

    xT_r = xT.rearrange("(dt p) s -> p dt s", p=P)          # [128, 16, S]
    wqd_r = wqd.rearrange("(dt p) q -> p dt q", p=P)        # [128, 16, 256]
    wkvd_r = wkvd.rearrange("(dt p) q -> p dt q", p=P)      # [128, 16, 128]
    wkr_r = wkr.rearrange("(dt p) q -> p dt q", p=P)        # [128, 16, 256]
    wqcat_r = wqcat.rearrange("(qt p) c -> p qt c", p=P)    # [128, 8, 512]
    wkup_r = wkup.rearrange("(kt p) c -> p kt c", p=P)      # [128, 4, 256]
    wvup_r = wvup.rearrange("(kt p) c -> p kt c", p=P)      # [128, 4, 512]
    wo_r = wo.rearrange("(kt p) d -> p kt d", p=P)          # [128, 4, 2048]
    o_r = o_part.rearrange("(st p) d -> p st d", p=P)       # [128, 16, 2048]

    with tile.TileContext(nc) as tc:
        with (
            tc.tile_pool(name="persist", bufs=1) as pp,
            tc.tile_pool(name="acts", bufs=2) as ap_,
            tc.tile_pool(name="latg", bufs=2) as lg,
            tc.tile_pool(name="rope", bufs=2) as rp,
            tc.tile_pool(name="attn", bufs=3) as atp,
            tc.tile_pool(name="recp", bufs=2) as rcp,
            tc.tile_pool(name="outp", bufs=2) as op_,
            tc.tile_pool(name="aoutp", bufs=2) as aop,
            tc.tile_pool(name="dram", bufs=1, space="DRAM") as dp,
            tc.tile_pool(name="psA", bufs=2, space="PSUM") as psA,
            tc.tile_pool(name="psS", bufs=3, space="PSUM") as psS,
            tc.tile_pool(name="psD", bufs=2, space="PSUM") as psD,
            tc.tile_pool(name="psO", bufs=1, space="PSUM") as psO,
        ):
            # ---------------- persistent tiles ----------------
            kT = [pp.tile([P, HLOC, CHUNK], BF16, name=f"kT{j}", tag=f"kT{j}")
                  for j in range(NCHUNK)]                     # per-chunk K^T
            vnat = [pp.tile([P, CHUNK // P, HLOC * HD], BF16, name=f"vn{j}", tag=f"vn{j}")
                    for j in range(NCHUNK)]                   # per-chunk V nat
            mask = pp.tile([P, P], BF16, tag="mask")
            ones = pp.tile([P, P], BF16, tag="ones")
            wo_t = pp.tile([P, HLOC, D], BF16, tag="wo")
            wqc_t = pp.tile([P, QR // P, HLOC * HD], BF16, tag="wqc")
            wku_t = pp.tile([P, KVR // P, HLOC * ND], BF16, tag="wku")
            wvu_t = pp.tile([P, KVR // P, HLOC * HD], BF16, tag="wvu")
            wkr_t = pp.tile([P, D // P, HLOC * RD], BF16, tag="wkr")
            wqd_t = pp.tile([P, D // P, QRL], BF16, tag="wqd")
            wkvd_t = pp.tile([P, D // P, KVRL], BF16, tag="wkvd")
            cos_t = pp.tile([P, S], BF16, tag="cos")
            sin_t = pp.tile([P, S], BF16, tag="sin")

            # DRAM bounce buffers for the latent AllGather, one per chunk
            bin_ = [dp.tile([P, 3 * CHUNK], BF16, name=f"bin{j}", tag=f"bin{j}")
                    for j in range(NCHUNK)]
            bout = [dp.tile([4 * P, 3 * CHUNK], BF16, name=f"bo{j}", tag=f"bo{j}")
                    for j in range(NCHUNK)]

            nc.vector.memset(ones[:], 1.0)
            # PE warm-up during the initial DMA ramp: ~3.5us of tiny matmuls
            # (only dep: the memset) un-throttle the HAM clock gate before the
            # first real matmul arrives
            wps = psA.tile([P, CHUNK], F32, name="warmps", tag="psA")
            for _ in range(155):
                nc.tensor.matmul(wps[0:64, 0:64], ones[:, 0:64],
                                 ones[:, 0:64], start=True, stop=True)

            def load_residents():
                """Emitted after chunk 0's critical prologue DMAs; ordered by
                first use in the main loop."""
                nc.sync.dma_start(wqc_t[:], wqcat_r[:])
                nc.sync.dma_start(wku_t[:], wkup_r[:])
                nc.sync.dma_start(wvu_t[:], wvup_r[:])
                nc.sync.dma_start(mask[:], maskd[:])
                for kt_ in range(HLOC):
                    nc.sync.dma_start(wo_t[:, kt_, :], wo_r[:, kt_, :])

            def o_proj(ic, aout, sts=range(CHUNK // P), final=False):
                """Project previous chunk's attention output; PE filler work
                staged across the next chunk's attention loop."""
                for st in sts:
                    osb = op_.tile([P, D], BF16, tag="osb")
                    for dc in range(D // CHUNK):
                        ps = psA.tile([P, CHUNK], F32, tag="psA")
                        for kt_ in range(HLOC):
                            nc.tensor.matmul(
                                ps[:], aout[:, kt_, P * st:P * (st + 1)],
                                wo_t[:, kt_, CHUNK * dc:CHUNK * (dc + 1)],
                                start=(kt_ == 0), stop=(kt_ == HLOC - 1))
                        if final and dc % 2 == 1:
                            # ScalarE is idle after the last exp
                            nc.scalar.copy(
                                osb[:, CHUNK * dc:CHUNK * (dc + 1)], ps[:])
                        else:
                            nc.vector.tensor_copy(
                                osb[:, CHUNK * dc:CHUNK * (dc + 1)], ps[:])
                        # per-dc store, alternated across two DMA rings so
                        # the final output drain runs on both concurrently
                        eng = nc.scalar if dc % 2 == 0 else nc.sync
                        eng.dma_start(
                            o_r[:, ic * (CHUNK // P) + st,
                                CHUNK * dc:CHUNK * (dc + 1)],
                            osb[:, CHUNK * dc:CHUNK * (dc + 1)])

            # ================= prologue: latent partials + k_pe =============
            # Everything here depends only on x and the down-proj weights.
            # Each chunk's qlat/kvlat column slice is DMAed to DRAM and
            # all-gathered across the 4-core batch group while later chunks
            # (and the main loop) keep the PE busy.
            for ic in range(NCHUNK):
                sl = slice(ic * CHUNK, (ic + 1) * CHUNK)
                if ic == 0:
                    # first matmul chain only needs the first wqd c-tile
                    nc.sync.dma_start(wqd_t[:, :, 0:P], wqd_r[:, :, 0:P])
                xc = ap_.tile([P, D // P, CHUNK], BF16, tag="xc")
                for dq in range(4):
                    nc.sync.dma_start(xc[:, 4 * dq:4 * (dq + 1), :],
                                      xT_r[:, 4 * dq:4 * (dq + 1), sl])
                if ic == 0:
                    nc.sync.dma_start(wqd_t[:, :, P:QRL], wqd_r[:, :, P:QRL])
                    nc.sync.dma_start(wkvd_t[:], wkvd_r[:])
                    nc.sync.dma_start(cos_t[:], cosr[:])
                    nc.sync.dma_start(sin_t[:], sinr[:])
                    nc.sync.dma_start(wkr_t[:], wkr_r[:])

                lat = ap_.tile([P, 3, CHUNK], BF16, tag="lat")
                for ci in range(2):                    # q_lat slice c-tiles
                    ps = psA.tile([P, CHUNK], F32, tag="psA")
                    for dt_ in range(D // P):
                        nc.tensor.matmul(
                            ps[:], wqd_t[:, dt_, P * ci:P * (ci + 1)],
                            xc[:, dt_, :],
                            start=(dt_ == 0), stop=(dt_ == D // P - 1))
                    nc.scalar.copy(lat[:, ci, :], ps[:])
                ps = psA.tile([P, CHUNK], F32, tag="psA")  # kv_lat slice
                for dt_ in range(D // P):
                    nc.tensor.matmul(
                        ps[:], wkvd_t[:, dt_, :], xc[:, dt_, :],
                        start=(dt_ == 0), stop=(dt_ == D // P - 1))
                nc.scalar.copy(lat[:, 2, :], ps[:])

                nc.scalar.dma_start(bin_[ic][:], lat[:])
                nc.gpsimd.collective_compute(
                    "AllGather", mybir.AluOpType.bypass,
                    replica_groups=GROUPS,
                    ins=[bin_[ic][:].opt()],
                    outs=[bout[ic][:].opt()])

                if ic == 0:
                    load_residents()

                # ---- k_pe: head pair a -> heads (2a, 2a+1) rope dims ----
                cos_c = cos_t[:, sl]
                sin_c = sin_t[:, sl]
                for a in range(2):
                    ps = psA.tile([P, CHUNK], F32, tag="psA")
                    for dt_ in range(D // P):
                        nc.tensor.matmul(
                            ps[:], wkr_t[:, dt_, P * a:P * (a + 1)],
                            xc[:, dt_, :],
                            start=(dt_ == 0), stop=(dt_ == D // P - 1))
                    raw = rp.tile([P, CHUNK], BF16, tag="kraw")
                    sh = rp.tile([P, CHUNK], BF16, tag="ksh")
                    scr = rp.tile([P, CHUNK], BF16, tag="kscr")
                    nc.scalar.copy(raw[:], ps[:])
                    # NeoX rotation: shifted halves within each 64-row block
                    for b in (0, 64):
                        nc.vector.tensor_copy(sh[b:b + 32, :],
                                              raw[b + 32:b + 64, :])
                        nc.vector.tensor_copy(sh[b + 32:b + 64, :],
                                              raw[b:b + 32, :])
                    nc.vector.tensor_tensor(sh[:], sh[:], sin_c,
                                            mybir.AluOpType.mult)
                    nc.vector.tensor_tensor(scr[:], raw[:], cos_c,
                                            mybir.AluOpType.mult)
                    nc.vector.tensor_tensor(kT[ic][64:128, 2 * a, :],
                                            scr[0:64, :], sh[0:64, :],
                                            mybir.AluOpType.add)
                    nc.vector.tensor_tensor(kT[ic][64:128, 2 * a + 1, :],
                                            scr[64:128, :], sh[64:128, :],
                                            mybir.AluOpType.add)

            # ================= main loop: consume gathered latents ==========
            for ic in range(NCHUNK):
                sl = slice(ic * CHUNK, (ic + 1) * CHUNK)
                cos_c = cos_t[:, sl]
                sin_c = sin_t[:, sl]

                # gathered latents -> SBUF. Rank r rows hold its q_lat
                # c-tiles (2r, 2r+1) then its kv_lat tile.
                qlat = lg.tile([P, QR // P, CHUNK], BF16, tag="qlat")
                kvlat = lg.tile([P, KVR // P, CHUNK], BF16, tag="kvlat")
                for r in range(4):
                    rs = slice(P * r, P * (r + 1))
                    nc.sync.dma_start(qlat[:, 2 * r:2 * r + 2, :],
                                      bout[ic][rs, 0:2 * CHUNK])
                    nc.sync.dma_start(kvlat[:, r, :],
                                      bout[ic][rs, 2 * CHUNK:3 * CHUNK])

                # ---- q heads: c-tile h = head h [nope64 | pe64] ----
                qTi = ap_.tile([P, HLOC, CHUNK], BF16, tag="qTi")
                for h in range(HLOC):
                    ps = psA.tile([P, CHUNK], F32, tag="psA")
                    for qt in range(QR // P):
                        nc.tensor.matmul(
                            ps[:], wqc_t[:, qt, P * h:P * (h + 1)],
                            qlat[:, qt, :],
                            start=(qt == 0), stop=(qt == QR // P - 1))
                    nc.scalar.copy(qTi[0:64, h, :], ps[0:64, :])
                    raw = rp.tile([P, CHUNK], BF16, tag="qraw")
                    sh = rp.tile([P, CHUNK], BF16, tag="qsh")
                    scr = rp.tile([P, CHUNK], BF16, tag="qscr")
                    nc.scalar.copy(raw[64:128, :], ps[64:128, :])
                    nc.vector.tensor_copy(sh[64:96, :], raw[96:128, :])
                    nc.vector.tensor_copy(sh[96:128, :], raw[64:96, :])
                    nc.vector.tensor_tensor(sh[64:128, :], sh[64:128, :],
                                            sin_c[64:128, :],
                                            mybir.AluOpType.mult)
                    nc.vector.tensor_tensor(scr[64:128, :], raw[64:128, :],
                                            cos_c[64:128, :],
                                            mybir.AluOpType.mult)
                    nc.vector.tensor_tensor(qTi[64:128, h, :],
                                            scr[64:128, :], sh[64:128, :],
                                            mybir.AluOpType.add)

                # ---- k_nope: head pair a -> heads (2a, 2a+1) nope dims ----
                for a in range(2):
                    ps = psA.tile([P, CHUNK], F32, tag="psA")
                    for kt_ in range(KVR // P):
                        nc.tensor.matmul(
                            ps[:], wku_t[:, kt_, P * a:P * (a + 1)],
                            kvlat[:, kt_, :],
                            start=(kt_ == 0), stop=(kt_ == KVR // P - 1))
                    if ic == 0:
                        # chunk 0's window is DVE-bound (rope chain); ScalarE
                        # has slack there (no overlapping attention exp)
                        nc.scalar.copy(kT[ic][0:64, 2 * a, :], ps[0:64, :])
                        nc.scalar.copy(kT[ic][0:64, 2 * a + 1, :],
                                       ps[64:128, :])
                    else:
                        nc.vector.tensor_copy(kT[ic][0:64, 2 * a, :],
                                              ps[0:64, :])
                        nc.vector.tensor_copy(kT[ic][0:64, 2 * a + 1, :],
                                              ps[64:128, :])

                # ---- v natural [CHUNK, 512] ----
                for st in range(CHUNK // P):
                    ps = psA.tile([P, HLOC * HD], F32, tag="psA")
                    for kt_ in range(KVR // P):
                        nc.tensor.matmul(
                            ps[:], kvlat[:, kt_, P * st:P * (st + 1)],
                            wvu_t[:, kt_, :],
                            start=(kt_ == 0), stop=(kt_ == KVR // P - 1))
                    if ic == 0:
                        nc.scalar.copy(vnat[ic][:, st, :], ps[:])
                    else:
                        nc.vector.tensor_copy(vnat[ic][:, st, :], ps[:])

                # ---- o-projection of the PREVIOUS chunk: PE filler that
                # covers this chunk's rope/DVE latency before attention
                if ic > 0:
                    o_proj(ic - 1, prev_aout, sts=(0, 1))

                # ---- attention for this query chunk (diagonal trimmed) ----
                aout = aop.tile([P, HLOC, CHUNK], BF16, tag="aout")
                for h in range(HLOC):
                    if ic > 0 and h in (2, 3):
                        # reserve late PE filler for the exp-paced tail
                        o_proj(ic - 1, prev_aout, sts=(h,))
                    psd = psD.tile([P, CHUNK], F32, tag="psD")
                    pso = psO.tile([P, CHUNK], F32, tag="psO")
                    nj = 4 * ic + 4            # total j-tiles incl. diagonal
                    for jt in range(nj):
                        jc, r = divmod(jt, 4)
                        diag = jc == ic
                        off = P * r if diag else 0      # first query col
                        n = CHUNK - off
                        first, last = jt == 0, jt == nj - 1
                        pss = psS.tile([P, CHUNK], F32, tag="psS")
                        nc.tensor.matmul(
                            pss[:, off:], kT[jc][:, h, P * r:P * (r + 1)],
                            qTi[:, h, off:], start=True, stop=True)
                        at = atp.tile([P, CHUNK], BF16, tag="attnT")
                        nc.scalar.activation(
                            at[:, off:], pss[:, off:],
                            mybir.ActivationFunctionType.Exp, scale=SCALE)
                        if diag:
                            nc.vector.tensor_tensor(
                                at[:, off:off + P], at[:, off:off + P],
                                mask[:], mybir.AluOpType.mult)
                        nc.tensor.matmul(
                            pso[:, off:], vnat[jc][:, r, HD * h:HD * (h + 1)],
                            at[:, off:], start=first, stop=last)
                        nc.tensor.matmul(psd[:, off:], ones[:], at[:, off:],
                                         start=first, stop=last)
                    rec = rcp.tile([P, CHUNK], F32, tag="recip")
                    nc.vector.reciprocal_approx_fast(rec[:], psd[:])
                    if ic == NCHUNK - 1 and h == HLOC - 1:
                        # final head gates o_proj(3): normalize per query
                        # slice so its st-blocks start earlier
                        for stq in range(CHUNK // P):
                            qs = slice(P * stq, P * (stq + 1))
                            nc.vector.tensor_tensor(
                                aout[:, h, qs], pso[:, qs], rec[:, qs],
                                mybir.AluOpType.mult)
                    else:
                        nc.vector.tensor_tensor(aout[:, h, :], pso[:], rec[:],
                                                mybir.AluOpType.mult)
                prev_aout = aout

            o_proj(NCHUNK - 1, prev_aout, final=True)
    nc.compile()
    return nc


_NC = None


def _get_nc():
    global _NC
    if _NC is None:
        _NC = _build()
    return _NC


def _host_prep(x, Wq_down, Wq_up, Wq_rope, Wkv_down, Wk_up, Wk_rope, Wv_up, Wo):
    """Build the 8 per-core input maps (all host-side layout prep)."""
    # rope tables, replicated to 128 partitions with NeoX sign baked into sin
    half = RD // 2
    inv_freq = 1.0 / (BASE ** (np.arange(half, dtype=np.float64) / half))
    ang = np.arange(S, dtype=np.float64)[None, :] * inv_freq[:, None]  # [32, S]
    cos32 = np.cos(ang)
    sin32 = np.sin(ang)
    cosr = np.tile(cos32, (4, 1)).astype(_BF16)                        # [128,S]
    sinr = np.concatenate([-sin32, sin32, -sin32, sin32], 0).astype(_BF16)

    # causal mask for the 128x128 diagonal block: key p visible to query c
    pidx = np.arange(P)[:, None]
    cidx = np.arange(P)[None, :]
    maskd = (pidx <= cidx).astype(_BF16)

    xT = [np.ascontiguousarray(x[b].T).astype(_BF16) for b in range(B)]

    in_maps = []
    for c in range(8):
        b, g = divmod(c, 4)
        heads = range(HLOC * g, HLOC * (g + 1))
        wqcat = np.empty((QR, HLOC * HD), np.float32)
        for i, h in enumerate(heads):
            wqcat[:, i * HD:i * HD + ND] = Wq_up[:, h * ND:(h + 1) * ND]
            wqcat[:, i * HD + ND:(i + 1) * HD] = Wq_rope[:, h * RD:(h + 1) * RD]
        in_maps.append({
            "xT": xT[b],
            "wqd": np.ascontiguousarray(
                Wq_down[:, g * QRL:(g + 1) * QRL]).astype(_BF16),
            "wkvd": np.ascontiguousarray(
                Wkv_down[:, g * KVRL:(g + 1) * KVRL]).astype(_BF16),
            "wkr": np.ascontiguousarray(
                Wk_rope[:, g * HLOC * RD:(g + 1) * HLOC * RD]).astype(_BF16),
            "wqcat": wqcat.astype(_BF16),
            "wkup": np.ascontiguousarray(
                Wk_up[:, g * HLOC * ND:(g + 1) * HLOC * ND]).astype(_BF16),
            "wvup": np.ascontiguousarray(
                Wv_up[:, g * HLOC * HD:(g + 1) * HLOC * HD]).astype(_BF16),
            "wo": np.ascontiguousarray(
                Wo[g * HLOC * HD:(g + 1) * HLOC * HD, :]).astype(_BF16),
            "cosr": cosr,
            "sinr": sinr,
            "maskd": maskd,
        })
    return in_maps


def kernel(x, Wq_down, Wq_up, Wq_rope, Wkv_down, Wk_up, Wk_rope, Wv_up, Wo,
           _trace=False, _trace_kwargs=None):
    x = np.asarray(x, dtype=np.float32)
    args = [np.asarray(a, dtype=np.float32) for a in
            (Wq_down, Wq_up, Wq_rope, Wkv_down, Wk_up, Wk_rope, Wv_up, Wo)]
    in_maps = _host_prep(x, *args)
    nc = _get_nc()
    res = run_bass_kernel_spmd(nc, in_maps, core_ids=list(range(8)),
                               trace=_trace, **(_trace_kwargs or {}))
    kernel._last_results = res
    out = np.zeros((B, S, D), np.float32)
    for c in range(8):
        out[c // 4] += res.results[c]["o_part"].astype(np.float32)
    return out
